# revision 4
# baseline (speedup 1.0000x reference)
"""Haar DWT-1D forward kernel for Trainium2, data-parallel over 8 NeuronCores.

The reference computes Lo = x @ matrix_low.T, Hi = x @ matrix_high.T where the
matrices are stride-2 banded Toeplitz with exactly two nonzeros per row:
    matrix_low[k, 2k] = a0,  matrix_low[k, 2k+1] = a1
    matrix_high[k, 2k] = b0, matrix_high[k, 2k+1] = b1
so the GEMM collapses to a pairwise (even, odd) combine:
    Lo[..., k] = a0 * x[..., 2k] + a1 * x[..., 2k+1]
    Hi[..., k] = b0 * x[..., 2k] + b1 * x[..., 2k+1]
The coefficients are read from the passed matrices at call time, so any
2-tap filter with this banded structure is handled.

Sharding: input (8, 64, 8192) -> core i gets batch slab i, (64, 8192).
On-chip each slab is viewed as 128 partitions x 4096 (row r, half h), the
pair dimension lives along the free axis (stride-2 access patterns).
"""

import numpy as np

import concourse.bacc as bacc
import concourse.bass as bass
import concourse.mybir as mybir
from concourse.bass_utils import run_bass_kernel_spmd
from concourse.tile import TileContext

N, C, L1 = 8, 64, 8192
L = L1 // 2
N_CORES = 8
ROWS = (N * C) // N_CORES  # 64 rows per core
F_TILE = 1024              # input columns per SBUF tile (of 4096 reshaped cols)

_FP32 = mybir.dt.float32

_program_cache: dict = {}


def _build_program(a0: float, a1: float, b0: float, b1: float) -> bass.Bass:
    nc = bacc.Bacc("TRN2")
    x = nc.dram_tensor("x", [ROWS, L1], _FP32, kind="ExternalInput")
    lo = nc.dram_tensor("lo", [ROWS, L], _FP32, kind="ExternalOutput")
    hi = nc.dram_tensor("hi", [ROWS, L], _FP32, kind="ExternalOutput")

    # Partition p = (r, h): row r of the slab, half h of its length-8192 line.
    xr = x[:].rearrange("r (h f) -> (r h) f", h=2)    # (128, 4096)
    lor = lo[:].rearrange("r (h f) -> (r h) f", h=2)  # (128, 2048)
    hir = hi[:].rearrange("r (h f) -> (r h) f", h=2)  # (128, 2048)

    n_tiles = xr.shape[1] // F_TILE
    G = F_TILE // 2

    with TileContext(nc) as tc:
        with (
            tc.tile_pool(name="xin", bufs=3) as xpool,
            tc.tile_pool(name="tmp", bufs=3) as tpool,
            tc.tile_pool(name="out", bufs=4) as opool,
        ):
            for j in range(n_tiles):
                xt = xpool.tile([128, F_TILE], _FP32, tag="x")
                nc.sync.dma_start(out=xt[:], in_=xr[:, j * F_TILE : (j + 1) * F_TILE])
                xv = xt[:].rearrange("p (k two) -> p k two", two=2)
                even, odd = xv[:, :, 0], xv[:, :, 1]

                # ec = a0 * even on ScalarE (ACT), frees DVE for the 2-tensor ops
                ec = tpool.tile([128, G], _FP32, tag="ec")
                nc.scalar.mul(ec[:], even, a0)
                lot = opool.tile([128, G], _FP32, tag="lo")
                nc.vector.scalar_tensor_tensor(
                    lot[:], odd, a1, ec[:], mybir.AluOpType.mult, mybir.AluOpType.add
                )
                if b0 == a0:
                    hc = ec
                else:
                    hc = tpool.tile([128, G], _FP32, tag="hc")
                    nc.scalar.mul(hc[:], even, b0)
                hit = opool.tile([128, G], _FP32, tag="hi")
                nc.vector.scalar_tensor_tensor(
                    hit[:], odd, b1, hc[:], mybir.AluOpType.mult, mybir.AluOpType.add
                )
                nc.sync.dma_start(out=lor[:, j * G : (j + 1) * G], in_=lot[:])
                nc.sync.dma_start(out=hir[:, j * G : (j + 1) * G], in_=hit[:])
    nc.finalize()
    return nc


def _get_program(a0, a1, b0, b1):
    key = (a0, a1, b0, b1)
    if key not in _program_cache:
        _program_cache[key] = _build_program(a0, a1, b0, b1)
    return _program_cache[key]


def kernel(input: np.ndarray, matrix_low: np.ndarray, matrix_high: np.ndarray, **_kw):
    x = np.asarray(input)
    assert x.shape == (N, C, L1), x.shape
    a0 = float(matrix_low[0, 0])
    a1 = float(matrix_low[0, 1])
    b0 = float(matrix_high[0, 0])
    b1 = float(matrix_high[0, 1])

    nc = _get_program(a0, a1, b0, b1)
    x = np.ascontiguousarray(x, dtype=np.float32)
    in_maps = [{"x": x[i]} for i in range(N_CORES)]
    res = run_bass_kernel_spmd(nc, in_maps, core_ids=list(range(N_CORES)))
    Lo = np.stack([res.results[i]["lo"] for i in range(N_CORES)])
    Hi = np.stack([res.results[i]["hi"] for i in range(N_CORES)])
    return (Lo, Hi)


# revision 5
# speedup vs baseline: 1.0119x; 1.0119x over previous
"""Haar DWT-1D forward kernel for Trainium2, data-parallel over 8 NeuronCores.

The reference computes Lo = x @ matrix_low.T, Hi = x @ matrix_high.T where the
matrices are stride-2 banded Toeplitz with exactly two nonzeros per row:
    matrix_low[k, 2k] = a0,  matrix_low[k, 2k+1] = a1
    matrix_high[k, 2k] = b0, matrix_high[k, 2k+1] = b1
so the GEMM collapses to a pairwise (even, odd) combine:
    Lo[..., k] = a0 * x[..., 2k] + a1 * x[..., 2k+1]
    Hi[..., k] = b0 * x[..., 2k] + b1 * x[..., 2k+1]
The coefficients are read from the passed matrices at call time, so any
2-tap filter with this banded structure is handled.

Sharding: input (8, 64, 8192) -> core i gets batch slab i, (64, 8192).
On-chip each slab is viewed as 128 partitions x 4096 (row r, half h); the
pair dimension lives along the free axis (stride-2 access patterns).

Dataflow per core: loads stream on the sync HWDGE ring, stores go out on
the gpsimd SWDGE ring so reads and writes use separate DMA queue rows.
When the filter is sum/difference shaped (a1 == a0, b1 == -b0) the compute
is S = e + o, D = e - o on VectorE (strided reads) followed by contiguous
scales on ScalarE; otherwise a general 2-tap path is used. Lo and Hi land
in one (128, 2, G) SBUF tile so a single DMA stores both bands.
"""

import numpy as np

import concourse.bacc as bacc
import concourse.bass as bass
import concourse.mybir as mybir
from concourse.bass_utils import run_bass_kernel_spmd
from concourse.tile import TileContext

N, C, L1 = 8, 64, 8192
L = L1 // 2
N_CORES = 8
ROWS = (N * C) // N_CORES  # 64 rows per core
F_TILE = 1024              # input columns per SBUF tile (of 4096 reshaped cols)

_FP32 = mybir.dt.float32

_program_cache: dict = {}


def _build_program(a0: float, a1: float, b0: float, b1: float) -> bass.Bass:
    nc = bacc.Bacc("TRN2")
    x = nc.dram_tensor("x", [ROWS, L1], _FP32, kind="ExternalInput")
    lohi = nc.dram_tensor("lohi", [2, ROWS, L], _FP32, kind="ExternalOutput")

    # Partition p = (r, h): row r of the slab, half h of its length-8192 line.
    xr = x[:].rearrange("r (h f) -> (r h) f", h=2)          # (128, 4096)
    yr = lohi[:].rearrange("b r (h f) -> (r h) b f", h=2)   # (128, 2, 2048)

    n_tiles = xr.shape[1] // F_TILE
    G = F_TILE // 2
    sumdiff = (a1 == a0) and (b1 == -b0)

    with TileContext(nc) as tc:
        with (
            tc.tile_pool(name="xin", bufs=3) as xpool,
            tc.tile_pool(name="tmp", bufs=3) as tpool,
            tc.tile_pool(name="out", bufs=3) as opool,
        ):
            for j in range(n_tiles):
                xt = xpool.tile([128, F_TILE], _FP32, tag="x")
                nc.sync.dma_start(out=xt[:], in_=xr[:, j * F_TILE : (j + 1) * F_TILE])
                xv = xt[:].rearrange("p (k two) -> p k two", two=2)
                even, odd = xv[:, :, 0], xv[:, :, 1]

                yt = opool.tile([128, 2, G], _FP32, tag="y")
                if sumdiff:
                    # S = e + o ; D = e - o on DVE, then contiguous scales on ACT
                    sd = tpool.tile([128, 2, G], _FP32, tag="sd")
                    nc.vector.tensor_add(out=sd[:, 0, :], in0=even, in1=odd)
                    nc.vector.tensor_sub(out=sd[:, 1, :], in0=even, in1=odd)
                    nc.scalar.mul(yt[:, 0, :], sd[:, 0, :], a0)
                    nc.scalar.mul(yt[:, 1, :], sd[:, 1, :], b0)
                else:
                    # General 2-tap: ec = a0*e (ACT), lo = a1*o + ec (DVE),
                    # hc = b0*e (ACT unless shared), hi = b1*o + hc (DVE)
                    ec = tpool.tile([128, G], _FP32, tag="ec")
                    nc.scalar.mul(ec[:], even, a0)
                    nc.vector.scalar_tensor_tensor(
                        yt[:, 0, :], odd, a1, ec[:],
                        mybir.AluOpType.mult, mybir.AluOpType.add,
                    )
                    if b0 == a0:
                        hc = ec
                    else:
                        hc = tpool.tile([128, G], _FP32, tag="hc")
                        nc.scalar.mul(hc[:], even, b0)
                    nc.vector.scalar_tensor_tensor(
                        yt[:, 1, :], odd, b1, hc[:],
                        mybir.AluOpType.mult, mybir.AluOpType.add,
                    )
                nc.gpsimd.dma_start(out=yr[:, :, j * G : (j + 1) * G], in_=yt[:])
    nc.finalize()
    return nc


def _get_program(a0, a1, b0, b1):
    key = (a0, a1, b0, b1)
    if key not in _program_cache:
        _program_cache[key] = _build_program(a0, a1, b0, b1)
    return _program_cache[key]


def kernel(input: np.ndarray, matrix_low: np.ndarray, matrix_high: np.ndarray, **_kw):
    x = np.asarray(input)
    assert x.shape == (N, C, L1), x.shape
    a0 = float(matrix_low[0, 0])
    a1 = float(matrix_low[0, 1])
    b0 = float(matrix_high[0, 0])
    b1 = float(matrix_high[0, 1])

    nc = _get_program(a0, a1, b0, b1)
    x = np.ascontiguousarray(x, dtype=np.float32)
    in_maps = [{"x": x[i]} for i in range(N_CORES)]
    res = run_bass_kernel_spmd(nc, in_maps, core_ids=list(range(N_CORES)))
    Lo = np.stack([res.results[i]["lohi"][0] for i in range(N_CORES)])
    Hi = np.stack([res.results[i]["lohi"][1] for i in range(N_CORES)])
    return (Lo, Hi)
